# revision 8
# baseline (speedup 1.0000x reference)
"""Trainium2 Bass kernel v6: fp8 DoubleRow with two-plane exact activations.

Math: the +/-1 weights are exact in fp8, and every scale in the network
is a power of two, so scales fold into the weights / final eviction
without rounding:
    z0 = W0^T x         (x = hi+lo split, exact to 2^-8)
    r0 = relu(z0)       h1 = hi/lo split of r0    (|z0| <~ 190 < 240)
    z1 = (c1 W1)^T h1   c1 = 2^-6 keeps |z1| <~ 135 < 240
    r1 = relu(z1)       h2 = hi/lo split of r1
    z2 = W2^T h2
    out = f z2          f = 4 s0 s1 s2 / c1
Each DoubleRow matmul contracts TWO k-tiles (K=256) in the time a bf16
matmul contracts one: one instruction for (hi_j, hi_j+1), one for
(lo_j, lo_j+1), sharing the same weight slice. Exact computation thus
matches bf16 speed; the win comes from skipping the lo plane on
LO_SKIP of the 32 k-tiles feeding layer 1 (HW-measured rel err
1.895e-2 vs the 2e-2 gate; error scales as 2.68e-2 * sqrt(n/32))
plus halved weight DMA traffic.

Instruction count: 512 + 1536 + 512 = 2560 matmuls vs 3072 for bf16
(measured ~574us vs 687us for the bf16 baseline, 216ns/matmul floor).
Startup/tail treatments carried over from the bf16 kernel iterations:
warmup matmuls during DMA queue boot, 6 interleaved head chains during
the x-paced window, dual-queue weight streaming, chunked output drain.
"""

from contextlib import ExitStack

import ml_dtypes
import numpy as np

P = 128
TOKENS = 8192
D_IN = 1024
D_H = 4096
D_OUT = 1024
N_CORES = 8
TOK_PER_CORE = TOKENS // N_CORES  # 1024
TOK_TILE = 512
NT = TOK_PER_CORE // TOK_TILE  # 2
L0_HEAD = 4
# Skipping a lo plane on an input k-tile of layer L costs identical error
# ((2.68e-2)^2/32 per tile, HW-calibrated) but saves 32 instructions on
# layer 1 (64 consumer chains) vs only 8 on layer 2 (16 chains) — so all
# skips go on h1. 16 tiles -> measured rel err 1.895e-2 vs the 2e-2 gate.
LO_SKIP = 16  # h1 k-tiles (of 32) whose lo plane is skipped in layer 1
LO_SKIP2 = 0  # h2 k-tiles (of 32) whose lo plane is skipped in layer 2
C1 = 2.0 ** -6

E4NP = ml_dtypes.float8_e4m3
BF16 = ml_dtypes.bfloat16

TRACE = False
TRACE_CORES = None
LAST_EXEC_TIME_NS = None
LAST_RESULT = None

_cache = {}


def _prune_dma_waits(nc, max_waits=1):
    """Drop transitively-implied waits from DMA instructions (see the
    bf16 kernel for the soundness argument)."""
    import bisect

    import bass_rust

    IN_ORDER_ENGINES = {
        "EngineType.PE",
        "EngineType.Activation",
        "EngineType.DVE",
        "EngineType.SP",
    }

    sem_hist = {}
    sem_cum = {}
    eng_clock = {}
    poisoned = set()

    def cc(sem, val):
        if sem in poisoned:
            return None
        hist = sem_hist.get(sem)
        if not hist or hist[0][-1] < val:
            return None
        return hist[1][bisect.bisect_left(hist[0], val)]

    def merge(dst, src):
        for k, v in src.items():
            if dst.get(k, 0) < v:
                dst[k] = v

    pruned = 0
    for bb in nc.m.functions[0].blocks:
        for inst in bb.instructions:
            si = inst.sync_info
            waits = list(si.on_wait or []) if si is not None else []
            ups = list(si.on_update or []) if si is not None else []
            is_dma = type(inst).__name__ == "InstDMACopy"

            clock = {}
            if not is_dma:
                prev = eng_clock.get(str(inst.engine))
                if prev is not None and str(inst.engine) in IN_ORDER_ENGINES:
                    merge(clock, prev)
            for w in waits:
                if w.wait_mode == "sem-ge-imm" and w.wait_value is not None:
                    c = cc(w.ant_name, w.wait_value)
                    if c is not None:
                        merge(clock, c)

            tname = type(inst).__name__
            if is_dma:
                cap = max_waits
            elif tname in ("InstDrain", "InstEventSemaphore", "InstCall",
                           "InstUnconditionalBranch", "InstISA"):
                cap = None
            else:
                cap = 2

            if cap is not None and len(waits) > cap:
                kept = list(waits)
                changed = True
                while len(kept) > cap and changed:
                    changed = False
                    for w in list(kept):
                        if w.wait_mode != "sem-ge-imm" or w.wait_value is None:
                            continue
                        implied = {}
                        provable = True
                        for o in kept:
                            if o is w:
                                continue
                            if o.wait_mode != "sem-ge-imm" or o.wait_value is None:
                                provable = False
                                break
                            c = cc(o.ant_name, o.wait_value)
                            if c is None:
                                provable = False
                                break
                            merge(implied, c)
                        if provable and implied.get(w.ant_name, 0) >= w.wait_value:
                            kept.remove(w)
                            pruned += 1
                            changed = True
                            break
                if len(kept) != len(waits):
                    inst.sync_info = bass_rust.SyncInfo(on_wait=kept, on_update=ups)

            own = {}
            for u in ups:
                if u.update_mode not in ("sem-inc", "sem-add-imm"):
                    poisoned.add(u.ant_name)
                    continue
                inc = 1 if u.update_mode == "sem-inc" else u.update_value
                if inc is None:
                    poisoned.add(u.ant_name)
                    continue
                sem = u.ant_name
                sem_cum[sem] = sem_cum.get(sem, 0) + inc
                own[sem] = sem_cum[sem]
            merge(clock, own)
            for sem, cum in own.items():
                vals, clocks = sem_hist.setdefault(sem, ([], []))
                vals.append(cum)
                clocks.append(clock)
            if not is_dma:
                eng_clock[str(inst.engine)] = clock
    return pruned


def _build(f_out):
    import concourse.mybir as mybir
    import concourse.tile as tile
    from concourse import bacc
    from concourse.alu_op_type import AluOpType

    nc = bacc.Bacc(
        "TRN2",
        target_bir_lowering=False,
        debug=False,
        enable_asserts=False,
        num_devices=N_CORES,
    )
    bf = mybir.dt.bfloat16
    f32 = mybir.dt.float32
    e4 = mybir.dt.float8e4
    T = TOK_PER_CORE
    DR = mybir.MatmulPerfMode.DoubleRow
    relu = mybir.ActivationFunctionType.Relu

    xhi_d = nc.dram_tensor("xhi", [D_IN // P, P, T], e4, kind="ExternalInput")
    xlo_d = nc.dram_tensor("xlo", [D_IN // P, P, T], e4, kind="ExternalInput")
    # packed [n_t, P, k_sub, P]: [n, p, j, c] = Wscaled[j*P+p, n*P+c]
    w0d = nc.dram_tensor("w0p", [D_H // P, P, D_IN // P, P], e4,
                         kind="ExternalInput")
    w1d = nc.dram_tensor("w1p", [D_H // P, P, D_H // P, P], e4,
                         kind="ExternalInput")
    w2d = nc.dram_tensor("w2p", [D_OUT // P, P, D_H // P, P], e4,
                         kind="ExternalInput")
    outt = nc.dram_tensor("outt", [D_OUT, T], f32, kind="ExternalOutput")

    with tile.TileContext(nc) as tc, ExitStack() as ctx:
        xpool = ctx.enter_context(tc.tile_pool(name="xp", bufs=1))
        h1pool = ctx.enter_context(tc.tile_pool(name="h1p", bufs=1))
        h2pool = ctx.enter_context(tc.tile_pool(name="h2p", bufs=1))
        w0pool = ctx.enter_context(tc.tile_pool(name="wp0", bufs=8))
        wpool = ctx.enter_context(tc.tile_pool(name="wp", bufs=4))
        opool = ctx.enter_context(tc.tile_pool(name="op", bufs=6))
        warmpool = ctx.enter_context(tc.tile_pool(name="wm", bufs=1))
        pspool = ctx.enter_context(tc.tile_pool(name="psp", bufs=8, space="PSUM"))

        # PE warmup during DMA cold-start.
        warm = warmpool.tile([P, TOK_TILE], bf, tag="warm", name="warm")
        nc.vector.memset(warm[:], 0.0)
        pw = pspool.tile([P, TOK_TILE], f32, tag="ps", name="psw")
        for i in range(14):
            nc.tensor.matmul(pw[:], warm[:, :P], warm[:], start=True, stop=True)

        # Activation planes as [P, k_sub, T] tiles (dim1 = k-tile index,
        # the DoubleRow plane axis).
        Xhi = xpool.tile([P, D_IN // P, T], e4, tag="xhi", name="Xhi")
        Xlo = xpool.tile([P, D_IN // P, T], e4, tag="xlo", name="Xlo")
        # hi planes on scalar, lo planes on gpsimd; the last pair rides
        # the sync queue behind the head weight strips (sync's first
        # slots must stay with the weights — the head chains touch all
        # head strips on the very first x pair).
        for j in range(D_IN // P - 1):
            nc.scalar.dma_start(out=Xhi[:, j, :], in_=xhi_d[j])
            nc.gpsimd.dma_start(out=Xlo[:, j, :], in_=xlo_d[j])

        H1hi = h1pool.tile([P, D_H // P, T], e4, tag="h1hi", name="H1hi")
        H1lo = h1pool.tile([P, D_H // P, T], e4, tag="h1lo", name="H1lo")
        H2hi = h2pool.tile([P, D_H // P, T], e4, tag="h2hi", name="H2hi")
        H2lo = h2pool.tile([P, D_H // P, T], e4, tag="h2lo", name="H2lo")

        def chain(ps, w, Hhi, Hlo, k_sub, t, lo_pairs):
            """Accumulate one output tile: K=256 DoubleRow pairs, hi plane
            always, lo plane for pairs < lo_pairs."""
            n_pairs = k_sub // 2
            insts = []
            for q in range(n_pairs):
                insts.append((q, Hhi))
                if q < lo_pairs:
                    insts.append((q, Hlo))
            tsl = slice(t * TOK_TILE, (t + 1) * TOK_TILE)
            for i, (q, Hsrc) in enumerate(insts):
                nc.tensor.matmul(
                    ps[:],
                    w[:, 2 * q : 2 * q + 2, :],
                    Hsrc[:, 2 * q : 2 * q + 2, tsl],
                    start=(i == 0),
                    stop=(i == len(insts) - 1),
                    perf_mode=DR,
                )

        def evict_hidden(Hhi, Hlo, n, t, ps, with_lo):
            tsl = slice(t * TOK_TILE, (t + 1) * TOK_TILE)
            nc.scalar.activation(Hhi[:, n, tsl], ps[:], relu, scale=1.0)
            if with_lo:
                nc.vector.scalar_tensor_tensor(
                    Hlo[:, n, tsl], ps[:], 0.0, Hhi[:, n, tsl],
                    AluOpType.max, AluOpType.subtract,
                )

        # ---- Layer 0: x (hi+lo) -> h1 ----
        k0 = D_IN // P  # 8
        n_h1 = D_H // P  # 32
        lo_keep_h1 = n_h1 - LO_SKIP  # h1 strips needing a lo plane

        head_w = []
        for n in range(L0_HEAD):
            w = w0pool.tile([P, k0, P], e4, tag="w0", name=f"w0_{n}")
            nc.sync.dma_start(out=w[:], in_=w0d[n])
            head_w.append(w)
        jl = D_IN // P - 1
        nc.sync.dma_start(out=Xhi[:, jl, :], in_=xhi_d[jl])
        nc.sync.dma_start(out=Xlo[:, jl, :], in_=xlo_d[jl])
        head_ps = [
            [pspool.tile([P, TOK_TILE], f32, tag="ps", name=f"psA_{n}_{t}")
             for t in range(NT)]
            for n in range(L0_HEAD)
        ]
        # j-pair innermost across the 6 head chains: 12 matmuls per
        # arriving x pair during the x-paced window.
        for q in range(k0 // 2):
            for n in range(L0_HEAD):
                for t in range(NT):
                    tsl = slice(t * TOK_TILE, (t + 1) * TOK_TILE)
                    nc.tensor.matmul(
                        head_ps[n][t][:],
                        head_w[n][:, 2 * q : 2 * q + 2, :],
                        Xhi[:, 2 * q : 2 * q + 2, tsl],
                        start=(q == 0), stop=False, perf_mode=DR,
                    )
                    nc.tensor.matmul(
                        head_ps[n][t][:],
                        head_w[n][:, 2 * q : 2 * q + 2, :],
                        Xlo[:, 2 * q : 2 * q + 2, tsl],
                        start=False, stop=(q == k0 // 2 - 1), perf_mode=DR,
                    )
        for n in range(L0_HEAD):
            for t in range(NT):
                evict_hidden(H1hi, H1lo, n, t, head_ps[n][t],
                             with_lo=(n < lo_keep_h1))

        for n in range(L0_HEAD, n_h1):
            w = w0pool.tile([P, k0, P], e4, tag="w0", name=f"w0_{n}")
            eng = nc.sync if (n < 8 or n % 2 == 0) else nc.gpsimd
            eng.dma_start(out=w[:], in_=w0d[n])
            for t in range(NT):
                ps = pspool.tile([P, TOK_TILE], f32, tag="ps", name=f"ps0_{n}_{t}")
                chain(ps, w, Xhi, Xlo, k0, t, lo_pairs=k0 // 2)
                evict_hidden(H1hi, H1lo, n, t, ps, with_lo=(n < lo_keep_h1))

        # ---- Layer 1: h1 -> h2 (lo plane skipped on last LO_SKIP k-tiles)
        k1 = D_H // P  # 32
        for n in range(D_H // P):
            w = wpool.tile([P, k1, P], e4, tag="w", name=f"w1_{n}")
            eng = nc.gpsimd if n % 2 == 0 else nc.sync
            eng.dma_start(out=w[:], in_=w1d[n])
            pss = [pspool.tile([P, TOK_TILE], f32, tag="ps", name=f"ps1_{n}_{t}")
                   for t in range(NT)]
            for t in range(NT):
                chain(pss[t], w, H1hi, H1lo, k1, t, lo_pairs=lo_keep_h1 // 2)
            for t in range(NT):
                evict_hidden(H2hi, H2lo, n, t, pss[t],
                             with_lo=(n < D_H // P - LO_SKIP2))

        # ---- Layer 2: h2 -> out ----
        CH = TOK_TILE // 2
        OUT_QS = [nc.sync, nc.gpsimd, nc.scalar]

        def evict_out(n, t, ps):
            for c in range(2):
                o = opool.tile([P, CH], f32, tag="o", name=f"o_{n}_{t}_{c}")
                sl = slice(c * CH, (c + 1) * CH)
                if c % 2 == 0:
                    nc.scalar.mul(o[:], ps[:, sl], f_out)
                else:
                    nc.vector.tensor_scalar_mul(o[:], ps[:, sl], f_out)
                eng = OUT_QS[(2 * t + c) % 3]
                eng.dma_start(
                    out=outt[
                        n * P : (n + 1) * P,
                        t * TOK_TILE + c * CH : t * TOK_TILE + (c + 1) * CH,
                    ],
                    in_=o[:],
                )

        k2 = D_H // P
        n_out = D_OUT // P
        for n in range(n_out):
            w = wpool.tile([P, k2, P], e4, tag="w", name=f"w2_{n}")
            eng = nc.gpsimd if n % 2 == 0 else nc.sync
            eng.dma_start(out=w[:], in_=w2d[n])
            lp2 = (k2 - LO_SKIP2) // 2
            if n == n_out - 1:
                # t-outer: the t=0 half drains while t=1 still computes.
                # No gpsimd DMAs on this strip: the gpsimd queue's slow
                # teardown DRAIN (~5us) then runs during compute instead
                # of after the last matmul. The last tile leaves in four
                # 64KB chunks alternating sync/scalar.
                LAST_QS = [nc.sync, nc.scalar]
                for t in range(NT):
                    ps = pspool.tile([P, TOK_TILE], f32, tag="ps",
                                     name=f"ps2_{n}_{t}")
                    chain(ps, w, H2hi, H2lo, k2, t, lo_pairs=lp2)
                    nch = 2 if t < NT - 1 else 4
                    CC = TOK_TILE // nch
                    # all evictions first (ACT/DVE in parallel), then the
                    # DMA issues — keeps ScalarE's ACTs off the critical
                    # path of its own descriptor pushes.
                    outs = []
                    for c in range(nch):
                        o = opool.tile([P, CC], f32, tag="o4",
                                       name=f"of_{t}_{c}")
                        sl = slice(c * CC, (c + 1) * CC)
                        if c % 2 == 0:
                            nc.scalar.mul(o[:], ps[:, sl], f_out)
                        else:
                            nc.vector.tensor_scalar_mul(o[:], ps[:, sl], f_out)
                        outs.append(o)
                    for c, o in enumerate(outs):
                        LAST_QS[c % 2].dma_start(
                            out=outt[
                                n * P : (n + 1) * P,
                                t * TOK_TILE + c * CC : t * TOK_TILE
                                + (c + 1) * CC,
                            ],
                            in_=o[:],
                        )
            else:
                pss = [pspool.tile([P, TOK_TILE], f32, tag="ps",
                                   name=f"ps2_{n}_{t}") for t in range(NT)]
                for t in range(NT):
                    chain(pss[t], w, H2hi, H2lo, k2, t, lo_pairs=lp2)
                for t in range(NT):
                    evict_out(n, t, pss[t])

    _prune_dma_waits(nc)
    nc.finalize()
    return nc


def _pack_w8(k, scale):
    """Bool [K, N] -> e4m3 +/-scale packed [N/P, P, K/P, P]:
    [n, p, j, c] = Wscaled[j*P+p, n*P+c]."""
    K, N = k.shape
    w = np.where(k, np.float32(scale), np.float32(-scale)).astype(E4NP)
    return np.ascontiguousarray(
        w.reshape(K // P, P, N // P, P).transpose(2, 1, 0, 3)
    )


def _enable_ntff_trace():
    import sys
    import types

    import concourse.bass_utils as bu

    bu.upload_artifacts = lambda tmpdir: tmpdir
    try:
        from antenv import axon_hooks
    except ImportError:
        import antenv

        axon_hooks = types.ModuleType("antenv.axon_hooks")
        _state = {"hook": None}
        axon_hooks.set_axon_ntff_profile_hook = lambda h: _state.__setitem__(
            "hook", h
        )
        axon_hooks.get_axon_ntff_profile_hook = lambda: _state["hook"]
        sys.modules["antenv.axon_hooks"] = axon_hooks
        antenv.axon_hooks = axon_hooks
    if axon_hooks.get_axon_ntff_profile_hook() is None:
        from trn_agent_boot.trn_boot import _ntff_profile_via_ctypes

        axon_hooks.set_axon_ntff_profile_hook(
            _ntff_profile_via_ctypes("/opt/axon/libaxon_pjrt.so")
        )


def kernel(x, k0, k1, k2, s0, s1, s2):
    global LAST_EXEC_TIME_NS, LAST_RESULT
    from concourse.bass_utils import run_bass_kernel_spmd

    if TRACE:
        _enable_ntff_trace()

    x = np.asarray(x, dtype=np.float32)
    s0 = float(np.asarray(s0))
    s1 = float(np.asarray(s1))
    s2 = float(np.asarray(s2))
    f_out = 4.0 * s0 * s1 * s2 / C1

    key = f_out
    if key not in _cache:
        _cache[key] = _build(f_out)
    nc = _cache[key]

    w0p = _pack_w8(np.asarray(k0), 1.0)
    w1p = _pack_w8(np.asarray(k1), C1)
    w2p = _pack_w8(np.asarray(k2), 1.0)

    in_maps = []
    for i in range(N_CORES):
        xs = np.ascontiguousarray(
            x[i * TOK_PER_CORE : (i + 1) * TOK_PER_CORE].T
        )  # [D_IN, T] f32
        xhi = xs.astype(E4NP)
        xlo = (xs - xhi.astype(np.float32)).astype(E4NP)
        in_maps.append(
            {
                "xhi": np.ascontiguousarray(
                    xhi.reshape(D_IN // P, P, TOK_PER_CORE)
                ),
                "xlo": np.ascontiguousarray(
                    xlo.reshape(D_IN // P, P, TOK_PER_CORE)
                ),
                "w0p": w0p,
                "w1p": w1p,
                "w2p": w2p,
            }
        )

    res = run_bass_kernel_spmd(
        nc, in_maps, list(range(N_CORES)), trace=TRACE, trace_cores=TRACE_CORES
    )
    LAST_EXEC_TIME_NS = res.exec_time_ns
    LAST_RESULT = res
    out = np.concatenate(
        [res.results[i]["outt"].T for i in range(N_CORES)], axis=0
    )
    return np.ascontiguousarray(out)
